# revision 28
# baseline (speedup 1.0000x reference)
"""Trainium2 Bass/Tile kernel: fused fp8-quantized multi-head causal attention.

Module: q/k/v = fp8(x) @ fp8(W) + b ; scores = (q k^T)/sqrt(64) with causal
mask (-1000 => exp underflows to exactly 0) ; out = softmax(scores) @ v @ W_O + b_O.

Sharding (8 NeuronCores, SPMD, no collectives):
  core c -> batch b = c // 4, head group hg = c % 4 (heads 4*hg .. 4*hg+3).
  Each core returns a partial [S, M] output (its 4 heads' contribution);
  the host sums the 4 partials per batch and adds b_O.

Host-side preprocessing: inputs/W_{Q,K,V} are quantized to fp8-e4m3 on the
host (bit-identical to the reference's jnp e4m3fn cast for |x| <= 240),
activations are uploaded transposed [M, S] so the contraction dim lands on
SBUF partitions, and weights are uploaded in partition-major layout so
every DMA moves >= 2 KiB contiguous rows. W_O is uploaded as bf16.

On-chip layout / dataflow per core:
  qT, kT   : [d'=256, S]  (d' = 4 heads x 64), DoubleRow fp8 matmuls,
             (x@W + b) * scale fused into the DVE psum eviction -> bf16
             (q's 1/8 score scale is exact in bf16: exponent-only).
  v        : [S, 4x(64+ones-col)] -> bf16 (ones col makes the z^T matmul
             accumulate the softmax denominator in row 64 for free).
  scores^T : one 2-bank psum [sk=128, 2, sq=512] per sk-chunk holds BOTH
             heads of an even/odd pair (lhsT/rhs at partition bases 0/64 ->
             row-group-packed concurrent matmuls, K=64 each). Diagonal-band
             tiles compute only the causally-live column range.
  pattern  : ONE exp per sk-chunk on ScalarE over both heads (2x1024-wide),
             plus a [128,128] triangular bf16 mask multiply on the band.
  z^T+denom: psum [65, sq] += v_h(lhsT [sk,65]) @ pattern.
  normalize: 1/denom via batched reciprocal_approx_accurate (~2 ULP);
             broadcast across 64 partitions with gpsimd partition_broadcast;
             zT_norm = zT * recip -> bf16 (DVE, fused with the eviction).
  out      : psum [s=128, m=512] = zt(lhsT [hd=128,s])^T @ W_O, 2 hd chunks,
             interleaved into the attention loop per sq window.
"""

import os
import sys

for _p in ("/opt/trn_rl_repo", os.path.expanduser("~/.axon_site/_ro/trn_rl_repo")):
    if os.path.isdir(_p) and _p not in sys.path:
        sys.path.insert(0, _p)

import ml_dtypes
import numpy as np

import concourse.bass as bass
import concourse.mybir as mybir
import concourse.tile as tile
from concourse import bacc
from concourse.bass_utils import run_bass_kernel_spmd

B, S, M, H, D = 2, 2048, 1024, 16, 64
HG = 4                 # heads per core
NCORES = 8
SQ = 512               # sq chunk width (one fp32 psum bank)
NSQ = S // SQ          # 4
NMC = M // 128         # 8 contraction chunks for projections
NSS = S // 128         # 16 s sub-chunks of 128

F8 = mybir.dt.float8e4
BF = mybir.dt.bfloat16
F32 = mybir.dt.float32
EXP = mybir.ActivationFunctionType.Exp
DR = mybir.MatmulPerfMode.DoubleRow

_f8 = ml_dtypes.float8_e4m3
_bf16 = ml_dtypes.bfloat16


def _build_nc():
    nc = bacc.Bacc(
        "TRN2", target_bir_lowering=False, debug=False, num_devices=NCORES
    )

    xq = nc.declare_dram_parameter("xq_t8", [M, S], F8, isOutput=False)
    xk = nc.declare_dram_parameter("xk_t8", [M, S], F8, isOutput=False)
    xv = nc.declare_dram_parameter("xv_t8", [M, S], F8, isOutput=False)
    wqkv = nc.declare_dram_parameter(
        "wqkv8", [128, 3 * NMC * HG * D], F8, isOutput=False
    )
    wo = nc.declare_dram_parameter("wo_bf", [128, 2 * M], BF, isOutput=False)
    bqk = nc.declare_dram_parameter("bqk", [128, 4], F32, isOutput=False)
    bv = nc.declare_dram_parameter("bv", [1, HG * D], F32, isOutput=False)
    out_p = nc.declare_dram_parameter("out_p", [S, M], F32, isOutput=True)

    with tile.TileContext(nc) as tc:
        with (
            tc.tile_pool(name="persist", bufs=1) as pers,
            tc.tile_pool(name="work", bufs=6) as work,
            tc.tile_pool(name="ppa", bufs=1, space="PSUM") as ppa,
            tc.tile_pool(name="pps", bufs=2, space="PSUM") as pps,
            tc.tile_pool(name="ppz", bufs=3, space="PSUM") as ppz,
        ):
            # ---- persistent SBUF tensors ----
            xq_sb = pers.tile([128, NMC, S], F8, tag="xq")
            xk_sb = pers.tile([128, NMC, S], F8, tag="xk")
            xv_sb = pers.tile([128, NMC, S], F8, tag="xv")
            wqkv_sb = pers.tile([128, 3, NMC, HG * D], F8, tag="wqkv")
            wq_sb, wk_sb, wv_sb = (wqkv_sb[:, i] for i in range(3))
            wo_sb = pers.tile([128, 2, M], BF, tag="wo")
            bqk_sb = pers.tile([128, 4], F32, tag="bqk")
            bq_sb, bk_sb = bqk_sb[:, 0:2], bqk_sb[:, 2:4]
            bv_sb = pers.tile([1, HG * D], F32, tag="bv")
            qt_sb = pers.tile([128, 2, S], BF, tag="qt")
            kt_sb = pers.tile([128, 2, S], BF, tag="kt")
            zt_sb = pers.tile([128, 2, S], BF, tag="zt")
            v_sb = pers.tile([128, NSS, HG, D + 1], BF, tag="v")
            trimask = pers.tile([128, 128], BF, tag="trimask")
            ones = pers.tile([1, SQ], F32, tag="ones")

            # ---- constants ----
            nc.gpsimd.memset(ones[:, :], 1.0)
            nc.gpsimd.memset(v_sb[:, :, :, D : D + 1], 1.0)
            # lower-triangular (inclusive) band mask: keep where row <= col
            nc.gpsimd.memset(trimask[:, :], 1.0)
            nc.gpsimd.affine_select(
                out=trimask[:, :],
                in_=trimask[:, :],
                compare_op=mybir.AluOpType.is_ge,
                fill=0.0,
                base=0,
                pattern=[[1, 128]],
                channel_multiplier=-1,
            )
            # warm the exp table set during the DMA phase
            expwarm = pers.tile([1, 1], F32, tag="expwarm")
            nc.scalar.activation(expwarm[:, :], ones[0:1, 0:1], EXP)

            # ---- input DMAs: weights/biases first, then 2 large DMAs per x ----
            nc.sync.dma_start(out=wqkv_sb[:, :, :, :], in_=wqkv[:, :])
            nc.sync.dma_start(out=bqk_sb[:, :], in_=bqk[:, :])
            nc.sync.dma_start(out=bv_sb[:, :], in_=bv[:, :])
            nc.sync.dma_start(out=wo_sb[:, :, :], in_=wo[:, :])
            # x loads. The critical first wave (q,k of s-cols 0:1024) is
            # issued from the idle vector/scalar queues (2 chains each) so
            # descriptor-issue doesn't serialize behind the weight DMAs on
            # sync; the rest go on sync in compute order, 2 chains per
            # (tensor, s-half).
            for x_sb, x_dram, q2 in ((xq_sb, xq, 0), (xq_sb, xq, 1),
                                     (xk_sb, xk, 0), (xk_sb, xk, 1)):
                nc.scalar.dma_start(
                    out=x_sb[:, 4 * q2 : 4 * q2 + 4, 0:1024],
                    in_=x_dram[512 * q2 : 512 * q2 + 512, 0:1024].rearrange(
                        "(c p) s -> p c s", p=128
                    ),
                )
            for x_sb, x_dram, g in ((xq_sb, xq, 1), (xk_sb, xk, 1),
                                    (xv_sb, xv, 0), (xv_sb, xv, 1)):
                for q2 in range(2):
                    nc.sync.dma_start(
                        out=x_sb[:, 4 * q2 : 4 * q2 + 4,
                                 1024 * g : 1024 * g + 1024],
                        in_=x_dram[512 * q2 : 512 * q2 + 512,
                                   1024 * g : 1024 * g + 1024].rearrange(
                            "(c p) s -> p c s", p=128
                        ),
                    )

            # ---- phase 1a: qT / kT projections ([d', s], fp8 DoubleRow) ----
            for t in range(NSQ):
                ssl = slice(SQ * t, SQ * t + SQ)
                for half in range(2):
                    dsl = slice(128 * half, 128 * half + 128)
                    for dst_sb, w_sb, x_sb, b_sb, scale in (
                        (qt_sb, wq_sb, xq_sb, bq_sb, 0.125),
                        (kt_sb, wk_sb, xk_sb, bk_sb, None),
                    ):
                        ps2p = pps.tile([128, 2, SQ], F32, tag="pps")
                        ps = ps2p[:, 0, :]
                        for mi in range(0, NMC, 2):
                            nc.tensor.matmul(
                                ps[:, :],
                                lhsT=w_sb[:, mi : mi + 2, dsl],
                                rhs=x_sb[:, mi : mi + 2, ssl],
                                start=(mi == 0),
                                stop=(mi == NMC - 2),
                                perf_mode=DR,
                            )
                        if scale is None:
                            nc.vector.tensor_scalar_add(
                                dst_sb[:, half, ssl], ps[:, :], b_sb[:, half : half + 1]
                            )
                        else:
                            nc.vector.tensor_scalar(
                                out=dst_sb[:, half, ssl],
                                in0=ps[:, :],
                                scalar1=b_sb[:, half : half + 1],
                                scalar2=scale,
                                op0=mybir.AluOpType.add,
                                op1=mybir.AluOpType.mult,
                            )

            # ---- phase 1b: v projection ([s, d'], ones col interleaved) ----
            for ss in range(NSS):
                psl = slice(128 * ss, 128 * ss + 128)
                ps2p = pps.tile([128, 2, SQ], F32, tag="pps")
                ps = ps2p[:, 0, :]
                for mi in range(0, NMC, 2):
                    nc.tensor.matmul(
                        ps[:, 0 : HG * D],
                        lhsT=xv_sb[:, mi : mi + 2, psl],
                        rhs=wv_sb[:, mi : mi + 2, :],
                        start=(mi == 0),
                        stop=False,
                        perf_mode=DR,
                    )
                nc.tensor.matmul(
                    ps[:, 0 : HG * D],
                    lhsT=ones[0:1, 0:128],
                    rhs=bv_sb[0:1, :],
                    start=False,
                    stop=True,
                )
                nc.vector.tensor_copy(
                    v_sb[:, ss, :, 0:D],
                    ps[:, 0 : HG * D].rearrange("p (g d) -> p g d", g=HG),
                )

            # ---- phase 2: attention, even/odd head pairs (row-group packed) ----
            for jq in range(NSQ):
                qsl = slice(SQ * jq, SQ * jq + SQ)
                nsk = 4 * (jq + 1)
                for c in range(2):  # head pair: heads (2c, 2c+1)
                    ps_z = [
                        ppz.tile([D + 1, SQ], F32, tag="ppz", name=f"psz{jq}_{c}_{u}")
                        for u in range(2)
                    ]
                    for si in range(nsk):
                        ksl = slice(128 * si, 128 * si + 128)
                        r = si - 4 * jq  # >=0 on diagonal-band tiles
                        w0 = 128 * r if r > 0 else 0  # fully-masked prefix
                        # both heads' scores into one 2-bank psum tile
                        ps2 = pps.tile([128, 2, SQ], F32, tag="pps")
                        for u in range(2):
                            hsl = slice(64 * u, 64 * u + 64)
                            nc.tensor.matmul(
                                ps2[:, u, w0:SQ],
                                lhsT=kt_sb[hsl, c, ksl],
                                rhs=qt_sb[hsl, c, SQ * jq + w0 : SQ * jq + SQ],
                                start=True,
                                stop=True,
                            )
                        p_bf = work.tile([128, 2, SQ], BF, tag="p")
                        nc.scalar.activation(
                            p_bf[:, :, w0:SQ], ps2[:, :, w0:SQ], EXP
                        )
                        if r >= 0:
                            for u in range(2):
                                nc.vector.tensor_mul(
                                    p_bf[:, u, w0 : w0 + 128],
                                    p_bf[:, u, w0 : w0 + 128],
                                    trimask[:, :],
                                )
                        for u in range(2):
                            h = 2 * c + u
                            nc.tensor.matmul(
                                ps_z[u][:, w0:SQ],
                                lhsT=v_sb[:, si, h, :],
                                rhs=p_bf[:, u, w0:SQ],
                                start=(si == 0),
                                stop=(si == nsk - 1),
                            )
                    # normalize both heads of the pair (~18-bit reciprocal is
                    # far below the bf16 pattern noise floor)
                    for u in range(2):
                        dn = work.tile([1, SQ], F32, tag="dn")
                        nc.vector.tensor_copy(dn[:, :], ps_z[u][D : D + 1, :])
                        recip = work.tile([1, SQ], F32, tag="recip")
                        nc.vector.reciprocal_approx_fast(
                            out=recip[:, :], in_=dn[:, :]
                        )
                        rb = work.tile([D, SQ], F32, tag="rb")
                        nc.gpsimd.partition_broadcast(rb[:, :], recip[0:1, :])
                        nc.vector.tensor_mul(
                            zt_sb[64 * u : 64 * u + 64, c, qsl],
                            ps_z[u][0:D, :],
                            rb[:, :],
                        )
                # ---- output projection for this jq's s-window ----
                for ss4 in range(4):
                    psl = slice(SQ * jq + 128 * ss4, SQ * jq + 128 * ss4 + 128)
                    for n in range(M // SQ):
                        nsl = slice(SQ * n, SQ * n + SQ)
                        if jq == NSQ - 1:
                            ps_o2 = pps.tile([128, 2, SQ], F32, tag="pps")
                            ps_o = ps_o2[:, 0, :]
                        else:
                            ps_o = ppa.tile([128, SQ], F32, tag="ppa")
                        for c in range(2):
                            nc.tensor.matmul(
                                ps_o[:, :],
                                lhsT=zt_sb[:, c, psl],
                                rhs=wo_sb[:, c, nsl],
                                start=(c == 0),
                                stop=(c == 1),
                            )
                        o_sb = work.tile([128, SQ], F32, tag="o")
                        nc.vector.tensor_copy(o_sb[:, :], ps_o[:, :])
                        nc.sync.dma_start(out=out_p[psl, nsl], in_=o_sb[:, :])

    if not nc.is_finalized():
        nc.finalize()
    return nc


_NC = None


def _get_nc():
    global _NC
    if _NC is None:
        _NC = _build_nc()
    return _NC


def _wpack(w):
    """[M, HG*D] -> partition-major [128, NMC*HG*D] (2 KiB contiguous rows)."""
    return np.ascontiguousarray(
        w.reshape(NMC, 128, HG * D).transpose(1, 0, 2).reshape(128, NMC * HG * D)
    )


def _make_in_maps(inputs):
    q8 = lambda a: np.asarray(a, np.float32).astype(_f8)
    xt = {}
    for name, key in (("xq_t8", "query_input"), ("xk_t8", "key_input"),
                      ("xv_t8", "value_input")):
        xt[name] = [np.ascontiguousarray(q8(inputs[key][b]).T) for b in range(B)]

    wq8 = q8(inputs["W_Q"])  # [H, M, D]
    wk8 = q8(inputs["W_K"])
    wv8 = q8(inputs["W_V"])
    wo = np.asarray(inputs["W_O"], np.float32)  # [H, D, M]

    in_maps = []
    for core in range(NCORES):
        b, hg = core // HG, core % HG
        hs = slice(HG * hg, HG * hg + HG)
        m = {
            "xq_t8": xt["xq_t8"][b],
            "xk_t8": xt["xk_t8"][b],
            "xv_t8": xt["xv_t8"][b],
            "wqkv8": np.concatenate(
                [
                    _wpack(w[hs].transpose(1, 0, 2).reshape(M, HG * D))
                    for w in (wq8, wk8, wv8)
                ],
                axis=1,
            ),
            "wo_bf": np.ascontiguousarray(
                wo[hs]
                .reshape(HG * D, M)
                .astype(_bf16)
                .reshape(2, 128, M)
                .transpose(1, 0, 2)
                .reshape(128, 2 * M)
            ),
            "bqk": np.ascontiguousarray(
                np.concatenate(
                    [
                        np.asarray(inputs[k], np.float32)[hs].reshape(2, 128).T
                        for k in ("b_Q", "b_K")
                    ],
                    axis=1,
                )
            ),
            "bv": np.asarray(inputs["b_V"], np.float32)[hs].reshape(1, HG * D).copy(),
        }
        in_maps.append(m)
    return in_maps


def _run(inputs, **kw):
    nc = _get_nc()
    in_maps = _make_in_maps(inputs)
    res = run_bass_kernel_spmd(nc, in_maps, list(range(NCORES)), **kw)
    out = np.zeros((B, S, M), np.float32)
    for core in range(NCORES):
        out[core // HG] += res.results[core]["out_p"]
    out += np.asarray(inputs["b_O"], np.float32)
    return out, res


def kernel(**inputs):
    out, _ = _run(inputs)
    return out


# revision 29
# speedup vs baseline: 1.0312x; 1.0312x over previous
"""Trainium2 Bass/Tile kernel: fused fp8-quantized multi-head causal attention.

Module: q/k/v = fp8(x) @ fp8(W) + b ; scores = (q k^T)/sqrt(64) with causal
mask (-1000 => exp underflows to exactly 0) ; out = softmax(scores) @ v @ W_O + b_O.

Sharding (8 NeuronCores, SPMD, no collectives):
  core c -> batch b = c // 4, head group hg = c % 4 (heads 4*hg .. 4*hg+3).
  Each core returns a partial [S, M] output (its 4 heads' contribution);
  the host sums the 4 partials per batch and adds b_O.

Host-side preprocessing: inputs/W_{Q,K,V} are quantized to fp8-e4m3 on the
host (bit-identical to the reference's jnp e4m3fn cast for |x| <= 240),
activations are uploaded transposed [M, S] so the contraction dim lands on
SBUF partitions, and weights are uploaded in partition-major layout so
every DMA moves >= 2 KiB contiguous rows. W_O is uploaded as bf16.

On-chip layout / dataflow per core:
  qT, kT   : [d'=256, S]  (d' = 4 heads x 64), DoubleRow fp8 matmuls,
             (x@W + b) * scale fused into the DVE psum eviction -> bf16
             (q's 1/8 score scale is exact in bf16: exponent-only).
  v        : [S, 4x(64+ones-col)] -> bf16 (ones col makes the z^T matmul
             accumulate the softmax denominator in row 64 for free).
  scores^T : one 2-bank psum [sk=128, 2, sq=512] per sk-chunk holds BOTH
             heads of an even/odd pair (lhsT/rhs at partition bases 0/64 ->
             row-group-packed concurrent matmuls, K=64 each). Diagonal-band
             tiles compute only the causally-live column range.
  pattern  : ONE exp per sk-chunk on ScalarE over both heads (2x1024-wide),
             plus a [128,128] triangular bf16 mask multiply on the band.
  z^T+denom: psum [65, sq] += v_h(lhsT [sk,65]) @ pattern.
  normalize: 1/denom via batched reciprocal_approx_accurate (~2 ULP);
             broadcast across 64 partitions with gpsimd partition_broadcast;
             zT_norm = zT * recip -> bf16 (DVE, fused with the eviction).
  out      : psum [s=128, m=512] = zt(lhsT [hd=128,s])^T @ W_O, 2 hd chunks,
             interleaved into the attention loop per sq window.
"""

import os
import sys

for _p in ("/opt/trn_rl_repo", os.path.expanduser("~/.axon_site/_ro/trn_rl_repo")):
    if os.path.isdir(_p) and _p not in sys.path:
        sys.path.insert(0, _p)

import ml_dtypes
import numpy as np

import concourse.bass as bass
import concourse.mybir as mybir
import concourse.tile as tile
from concourse import bacc
from concourse.bass_utils import run_bass_kernel_spmd

B, S, M, H, D = 2, 2048, 1024, 16, 64
HG = 4                 # heads per core
NCORES = 8
SQ = 512               # sq chunk width (one fp32 psum bank)
NSQ = S // SQ          # 4
NMC = M // 128         # 8 contraction chunks for projections
NSS = S // 128         # 16 s sub-chunks of 128

F8 = mybir.dt.float8e4
BF = mybir.dt.bfloat16
F32 = mybir.dt.float32
EXP = mybir.ActivationFunctionType.Exp
DR = mybir.MatmulPerfMode.DoubleRow

_f8 = ml_dtypes.float8_e4m3
_bf16 = ml_dtypes.bfloat16


def _build_nc():
    nc = bacc.Bacc(
        "TRN2", target_bir_lowering=False, debug=False, num_devices=NCORES
    )

    xq = nc.declare_dram_parameter("xq_t8", [M, S], F8, isOutput=False)
    xk = nc.declare_dram_parameter("xk_t8", [M, S], F8, isOutput=False)
    xv = nc.declare_dram_parameter("xv_t8", [M, S], F8, isOutput=False)
    wqkv = nc.declare_dram_parameter(
        "wqkv8", [128, 3 * NMC * HG * D], F8, isOutput=False
    )
    wo = nc.declare_dram_parameter("wo_bf", [128, 2 * M], BF, isOutput=False)
    bqk = nc.declare_dram_parameter("bqk", [128, 4], F32, isOutput=False)
    bv = nc.declare_dram_parameter("bv", [1, HG * D], F32, isOutput=False)
    out_p = nc.declare_dram_parameter("out_p", [S, M], F32, isOutput=True)

    with tile.TileContext(nc) as tc:
        with (
            tc.tile_pool(name="persist", bufs=1) as pers,
            tc.tile_pool(name="work", bufs=6) as work,
            tc.tile_pool(name="ppa", bufs=1, space="PSUM") as ppa,
            tc.tile_pool(name="pps", bufs=2, space="PSUM") as pps,
            tc.tile_pool(name="ppz", bufs=3, space="PSUM") as ppz,
        ):
            # ---- persistent SBUF tensors ----
            xq_sb = pers.tile([128, NMC, S], F8, tag="xq")
            xk_sb = pers.tile([128, NMC, S], F8, tag="xk")
            xv_sb = pers.tile([128, NMC, S], F8, tag="xv")
            wqkv_sb = pers.tile([128, 3, NMC, HG * D], F8, tag="wqkv")
            wq_sb, wk_sb, wv_sb = (wqkv_sb[:, i] for i in range(3))
            wo_sb = pers.tile([128, 2, M], BF, tag="wo")
            bqk_sb = pers.tile([128, 4], F32, tag="bqk")
            bq_sb, bk_sb = bqk_sb[:, 0:2], bqk_sb[:, 2:4]
            bv_sb = pers.tile([1, HG * D], F32, tag="bv")
            qt_sb = pers.tile([128, 2, S], BF, tag="qt")
            kt_sb = pers.tile([128, 2, S], BF, tag="kt")
            zt_sb = pers.tile([128, 2, S], BF, tag="zt")
            v_sb = pers.tile([128, NSS, HG, D + 1], BF, tag="v")
            trimask = pers.tile([128, 128], BF, tag="trimask")
            ones = pers.tile([1, SQ], F32, tag="ones")

            # ---- constants ----
            nc.gpsimd.memset(ones[:, :], 1.0)
            nc.gpsimd.memset(v_sb[:, :, :, D : D + 1], 1.0)
            # lower-triangular (inclusive) band mask: keep where row <= col
            nc.gpsimd.memset(trimask[:, :], 1.0)
            nc.gpsimd.affine_select(
                out=trimask[:, :],
                in_=trimask[:, :],
                compare_op=mybir.AluOpType.is_ge,
                fill=0.0,
                base=0,
                pattern=[[1, 128]],
                channel_multiplier=-1,
            )
            # warm the exp table set during the DMA phase
            expwarm = pers.tile([1, 1], F32, tag="expwarm")
            nc.scalar.activation(expwarm[:, :], ones[0:1, 0:1], EXP)

            # ---- input DMAs: weights/biases first, then 2 large DMAs per x ----
            W1 = NMC * HG * D
            for i in range(3):  # separate chains: wq, wk, wv in parallel
                nc.sync.dma_start(
                    out=wqkv_sb[:, i, :, :], in_=wqkv[:, W1 * i : W1 * i + W1]
                )
            nc.sync.dma_start(out=bqk_sb[:, :], in_=bqk[:, :])
            nc.sync.dma_start(out=bv_sb[:, :], in_=bv[:, :])
            nc.sync.dma_start(out=wo_sb[:, :, :], in_=wo[:, :])
            # x loads. The critical first wave (q,k of s-cols 0:1024) is
            # issued from the idle vector/scalar queues (2 chains each) so
            # descriptor-issue doesn't serialize behind the weight DMAs on
            # sync; the rest go on sync in compute order, 2 chains per
            # (tensor, s-half).
            # finest first wave: q,k s-cols 0:512 (proj t=0) on the scalar
            # queue, then widening windows; rest on sync in compute order
            for x_sb, x_dram, q2, ssl0, ssl1 in (
                (xq_sb, xq, 0, 0, 512), (xq_sb, xq, 1, 0, 512),
                (xk_sb, xk, 0, 0, 512), (xk_sb, xk, 1, 0, 512),
                (xq_sb, xq, 0, 512, 1024), (xq_sb, xq, 1, 512, 1024),
                (xk_sb, xk, 0, 512, 1024), (xk_sb, xk, 1, 512, 1024),
            ):
                nc.scalar.dma_start(
                    out=x_sb[:, 4 * q2 : 4 * q2 + 4, ssl0:ssl1],
                    in_=x_dram[512 * q2 : 512 * q2 + 512, ssl0:ssl1].rearrange(
                        "(c p) s -> p c s", p=128
                    ),
                )
            for x_sb, x_dram, g in ((xq_sb, xq, 1), (xk_sb, xk, 1),
                                    (xv_sb, xv, 0), (xv_sb, xv, 1)):
                for q2 in range(2):
                    nc.sync.dma_start(
                        out=x_sb[:, 4 * q2 : 4 * q2 + 4,
                                 1024 * g : 1024 * g + 1024],
                        in_=x_dram[512 * q2 : 512 * q2 + 512,
                                   1024 * g : 1024 * g + 1024].rearrange(
                            "(c p) s -> p c s", p=128
                        ),
                    )

            # ---- phase 1a: qT / kT projections ([d', s], fp8 DoubleRow) ----
            for t in range(NSQ):
                ssl = slice(SQ * t, SQ * t + SQ)
                for half in range(2):
                    dsl = slice(128 * half, 128 * half + 128)
                    for dst_sb, w_sb, x_sb, b_sb, scale in (
                        (qt_sb, wq_sb, xq_sb, bq_sb, 0.125),
                        (kt_sb, wk_sb, xk_sb, bk_sb, None),
                    ):
                        ps2p = pps.tile([128, 2, SQ], F32, tag="pps")
                        ps = ps2p[:, 0, :]
                        for mi in range(0, NMC, 2):
                            nc.tensor.matmul(
                                ps[:, :],
                                lhsT=w_sb[:, mi : mi + 2, dsl],
                                rhs=x_sb[:, mi : mi + 2, ssl],
                                start=(mi == 0),
                                stop=(mi == NMC - 2),
                                perf_mode=DR,
                            )
                        if scale is None:
                            nc.vector.tensor_scalar_add(
                                dst_sb[:, half, ssl], ps[:, :], b_sb[:, half : half + 1]
                            )
                        else:
                            nc.vector.tensor_scalar(
                                out=dst_sb[:, half, ssl],
                                in0=ps[:, :],
                                scalar1=b_sb[:, half : half + 1],
                                scalar2=scale,
                                op0=mybir.AluOpType.add,
                                op1=mybir.AluOpType.mult,
                            )

            # ---- phase 1b: v projection ([s, d'], ones col interleaved) ----
            for ss in range(NSS):
                psl = slice(128 * ss, 128 * ss + 128)
                ps2p = pps.tile([128, 2, SQ], F32, tag="pps")
                ps = ps2p[:, 0, :]
                for mi in range(0, NMC, 2):
                    nc.tensor.matmul(
                        ps[:, 0 : HG * D],
                        lhsT=xv_sb[:, mi : mi + 2, psl],
                        rhs=wv_sb[:, mi : mi + 2, :],
                        start=(mi == 0),
                        stop=False,
                        perf_mode=DR,
                    )
                nc.tensor.matmul(
                    ps[:, 0 : HG * D],
                    lhsT=ones[0:1, 0:128],
                    rhs=bv_sb[0:1, :],
                    start=False,
                    stop=True,
                )
                nc.vector.tensor_copy(
                    v_sb[:, ss, :, 0:D],
                    ps[:, 0 : HG * D].rearrange("p (g d) -> p g d", g=HG),
                )

            # ---- phase 2: attention, even/odd head pairs (row-group packed) ----
            for jq in range(NSQ):
                qsl = slice(SQ * jq, SQ * jq + SQ)
                nsk = 4 * (jq + 1)
                for c in range(2):  # head pair: heads (2c, 2c+1)
                    ps_z = [
                        ppz.tile([D + 1, SQ], F32, tag="ppz", name=f"psz{jq}_{c}_{u}")
                        for u in range(2)
                    ]
                    for si in range(nsk):
                        ksl = slice(128 * si, 128 * si + 128)
                        r = si - 4 * jq  # >=0 on diagonal-band tiles
                        w0 = 128 * r if r > 0 else 0  # fully-masked prefix
                        # both heads' scores into one 2-bank psum tile
                        ps2 = pps.tile([128, 2, SQ], F32, tag="pps")
                        for u in range(2):
                            hsl = slice(64 * u, 64 * u + 64)
                            nc.tensor.matmul(
                                ps2[:, u, w0:SQ],
                                lhsT=kt_sb[hsl, c, ksl],
                                rhs=qt_sb[hsl, c, SQ * jq + w0 : SQ * jq + SQ],
                                start=True,
                                stop=True,
                            )
                        p_bf = work.tile([128, 2, SQ], BF, tag="p")
                        nc.scalar.activation(
                            p_bf[:, :, w0:SQ], ps2[:, :, w0:SQ], EXP
                        )
                        if r >= 0:
                            for u in range(2):
                                nc.vector.tensor_mul(
                                    p_bf[:, u, w0 : w0 + 128],
                                    p_bf[:, u, w0 : w0 + 128],
                                    trimask[:, :],
                                )
                        for u in range(2):
                            h = 2 * c + u
                            nc.tensor.matmul(
                                ps_z[u][:, w0:SQ],
                                lhsT=v_sb[:, si, h, :],
                                rhs=p_bf[:, u, w0:SQ],
                                start=(si == 0),
                                stop=(si == nsk - 1),
                            )
                    # normalize both heads of the pair (~18-bit reciprocal is
                    # far below the bf16 pattern noise floor)
                    for u in range(2):
                        dn = work.tile([1, SQ], F32, tag="dn")
                        nc.vector.tensor_copy(dn[:, :], ps_z[u][D : D + 1, :])
                        recip = work.tile([1, SQ], F32, tag="recip")
                        nc.vector.reciprocal_approx_fast(
                            out=recip[:, :], in_=dn[:, :]
                        )
                        rb = work.tile([D, SQ], F32, tag="rb")
                        nc.gpsimd.partition_broadcast(rb[:, :], recip[0:1, :])
                        nc.vector.tensor_mul(
                            zt_sb[64 * u : 64 * u + 64, c, qsl],
                            ps_z[u][0:D, :],
                            rb[:, :],
                        )
                # ---- output projection for this jq's s-window ----
                for ss4 in range(4):
                    psl = slice(SQ * jq + 128 * ss4, SQ * jq + 128 * ss4 + 128)
                    for n in range(M // SQ):
                        nsl = slice(SQ * n, SQ * n + SQ)
                        if jq == NSQ - 1:
                            ps_o2 = pps.tile([128, 2, SQ], F32, tag="pps")
                            ps_o = ps_o2[:, 0, :]
                        else:
                            ps_o = ppa.tile([128, SQ], F32, tag="ppa")
                        for c in range(2):
                            nc.tensor.matmul(
                                ps_o[:, :],
                                lhsT=zt_sb[:, c, psl],
                                rhs=wo_sb[:, c, nsl],
                                start=(c == 0),
                                stop=(c == 1),
                            )
                        o_sb = work.tile([128, SQ], F32, tag="o")
                        nc.vector.tensor_copy(o_sb[:, :], ps_o[:, :])
                        nc.sync.dma_start(out=out_p[psl, nsl], in_=o_sb[:, :])

    if not nc.is_finalized():
        nc.finalize()
    return nc


_NC = None


def _get_nc():
    global _NC
    if _NC is None:
        _NC = _build_nc()
    return _NC


def _wpack(w):
    """[M, HG*D] -> partition-major [128, NMC*HG*D] (2 KiB contiguous rows)."""
    return np.ascontiguousarray(
        w.reshape(NMC, 128, HG * D).transpose(1, 0, 2).reshape(128, NMC * HG * D)
    )


def _make_in_maps(inputs):
    q8 = lambda a: np.asarray(a, np.float32).astype(_f8)
    xt = {}
    for name, key in (("xq_t8", "query_input"), ("xk_t8", "key_input"),
                      ("xv_t8", "value_input")):
        xt[name] = [np.ascontiguousarray(q8(inputs[key][b]).T) for b in range(B)]

    wq8 = q8(inputs["W_Q"])  # [H, M, D]
    wk8 = q8(inputs["W_K"])
    wv8 = q8(inputs["W_V"])
    wo = np.asarray(inputs["W_O"], np.float32)  # [H, D, M]

    in_maps = []
    for core in range(NCORES):
        b, hg = core // HG, core % HG
        hs = slice(HG * hg, HG * hg + HG)
        m = {
            "xq_t8": xt["xq_t8"][b],
            "xk_t8": xt["xk_t8"][b],
            "xv_t8": xt["xv_t8"][b],
            "wqkv8": np.concatenate(
                [
                    _wpack(w[hs].transpose(1, 0, 2).reshape(M, HG * D))
                    for w in (wq8, wk8, wv8)
                ],
                axis=1,
            ),
            "wo_bf": np.ascontiguousarray(
                wo[hs]
                .reshape(HG * D, M)
                .astype(_bf16)
                .reshape(2, 128, M)
                .transpose(1, 0, 2)
                .reshape(128, 2 * M)
            ),
            "bqk": np.ascontiguousarray(
                np.concatenate(
                    [
                        np.asarray(inputs[k], np.float32)[hs].reshape(2, 128).T
                        for k in ("b_Q", "b_K")
                    ],
                    axis=1,
                )
            ),
            "bv": np.asarray(inputs["b_V"], np.float32)[hs].reshape(1, HG * D).copy(),
        }
        in_maps.append(m)
    return in_maps


def _run(inputs, **kw):
    nc = _get_nc()
    in_maps = _make_in_maps(inputs)
    res = run_bass_kernel_spmd(nc, in_maps, list(range(NCORES)), **kw)
    out = np.zeros((B, S, M), np.float32)
    for core in range(NCORES):
        out[core // HG] += res.results[core]["out_p"]
    out += np.asarray(inputs["b_O"], np.float32)
    return out, res


def kernel(**inputs):
    out, _ = _run(inputs)
    return out
